# revision 35
# baseline (speedup 1.0000x reference)
"""GAT-style attention (gnn_message_passing) Trainium2 kernel, 8-core row-parallel.

Math (algebraically identical to the reference masked-softmax attention):
  E = relu(h @ P)                 [N,3]
  W' = max(exp(E - 4ln2), 1/16)   (= exp(relu(E))/16, fp16-safe range)
  denom'[i,k] = sum_j A[i,j] W'[j,k]   (k=3 slot sums ones -> rowsum[i])
  R'[i,k] = rowsum[i] / denom'[i,k]
  ct[j,i]  = sum_k W'[j,k] R'[i,k] = rowsum[i] * C[i,j]
  mt[j,i]  = A[i,j] * ct[j,i]
  out[i,:] = sum_j mt[j,i] h[j,:]

Two SPMD programs (cost-modeled collectives are ~15us fixed -> too slow; the
tiny [4096,3] W matrix crosses cores via a host gather between programs):
  P1 (per core): W'-shard [512,3] from host-transposed h-shard (fp16 — fp8
      h.T fails the error budget through the exponential).
  host: concat W'-shards; build wt [3,N] / w4 (W'|ones) layouts; cast
      A-shard.T to fp8 (binary, exact); split h into fp8 hi/lo halves
      (h = h_hi + h_lo, each e4m3; lossless-ish re-encoding).
  P2 (per core): denominators stream with the A.T pieces (at8-stationary
      matmuls, one PSUM accumulation "super-group"), R' chain, then 16
      jc-pair sweeps:
        ct pair (fp16 matmuls, [128,1024] PSUM)
        mt16 = at8 * ct          (DVE, the only full-size PSUM touch)
        mt_hi8 = fp8(mt16)       (ACT copies, a couple on POOL for balance)
        mt_lo8 = mt16 - mt_hi8   (DVE/POOL split)
        psO[ic] += DoubleRow fp8 matmuls: mt_hi.T@h_hi + mt_hi.T@h_lo
                   + mt_lo.T@h_hi   (3-term split => 0.3% rel err, 4x
                   cheaper than fp16 per the 0.5 cycles/row DR rate)
      Warm-up matmuls during the initial load defeat the PE clock ramp.
"""

import numpy as np
import ml_dtypes

import concourse.bass as bass
import concourse.mybir as mybir
import concourse.tile as tile
from concourse import bacc
from concourse import bass_utils

N = 4096
D = 512
H = 3
NCORES = 8
SH = N // NCORES          # 512 output rows per core
JC = N // 128             # 32 j-chunks
IC = SH // 128            # 4 i-chunks
DC = D // 128             # 4 d-chunks
NP = JC // 2              # 16 jc-pairs
F8 = mybir.dt.float8e4
F16 = mybir.dt.float16
F32 = mybir.dt.float32
LN2x4 = float(4.0 * np.log(2.0))   # W scaled by 2^-4 to stay in fp16 range
NP_F8 = ml_dtypes.float8_e4m3
DR = mybir.MatmulPerfMode.DoubleRow


def _body1(tc, hst_in, p_in, w_out):
    """P1: W'-shard [SH,3] from hst [128, IC*DC*128] (h-shard.T, jc-major:
    hst[:, jc, dc, :] = h.T d-chunk dc for j-chunk jc). Loaded and computed
    in per-jc waves so each wave's exp/max/w_out DMA overlaps the next
    wave's hst transfer + completion semaphore."""
    nc = tc.nc
    with (
        tc.tile_pool(name="sb1", bufs=1) as sb,
        tc.tile_pool(name="ps1", bufs=1, space="PSUM") as ps,
    ):
        hst = sb.tile([128, IC * DC * 128], F16, tag="hst")
        p16 = sb.tile([128, DC * H], F16, tag="p16")
        wsE = sb.tile([128, IC * H], F16, tag="wsE")
        ebias = sb.tile([128, 1], F32, tag="ebias")
        nc.gpsimd.dma_start(out=p16[:], in_=p_in)
        nc.vector.memset(ebias[:], -LN2x4)
        hst_v = hst[:].rearrange("p (g x) -> g p x", g=2)
        hin_v = hst_in.rearrange("p (g x) -> g p x", g=2)
        for g in range(2):
            nc.sync.dma_start(out=hst_v[g], in_=hin_v[g])

        # one PSUM tile spanning 4 banks: E group per jc, single exp at the end
        psE = ps.tile([128, IC * 512], F32, tag="psE", name="psE")
        for jc in range(IC):
            for dc in range(DC):
                nc.tensor.matmul(
                    psE[:, jc * 512: jc * 512 + H],
                    hst[:, (jc * DC + dc) * 128: (jc * DC + dc + 1) * 128],
                    p16[:, dc * H:(dc + 1) * H],
                    start=(dc == 0),
                    stop=(dc == DC - 1),
                )
        nc.scalar.activation(
            wsE[:].rearrange("p (jc k) -> p jc k", k=H),
            psE[:].rearrange("p (jc x) -> p jc x", x=512)[:, :, 0:H],
            mybir.ActivationFunctionType.Exp,
            bias=ebias[:], scale=1.0,
        )
        nc.vector.tensor_scalar_max(wsE[:], wsE[:], 0.0625)
        nc.sync.dma_start(out=w_out, in_=wsE[:])


def _body2(tc, a8_in, hh_in, hl_in, wt_in, w4_in, id_in, out):
    """P2: denominators + R' chain + 16 uniform sweep units with 3-term fp8
    DoubleRow aggregation.

    a8_in is A-shard.T fp8 packed [p, ihalf, jc, 256]: the i (output-row)
    space is split into two halves of 256. A unit = (ihalf, jc-quad): a
    [128, 1024] slab covering 4 j-chunks x one i-half = 2 DR pair-planes x
    2 ic blocks. A-half units depend only on the A-half denominators (the
    first 1 MB of at8), so the sweep starts ~3us earlier than a full-R
    schedule; B-half units start once the full at8 has landed.

    PSUM budget trick: the denominator accumulator psD2 and the R-transpose
    scratch psRT live inside psO[2]/psO[3]'s banks (bitcast slices). They
    are fully consumed before the first B-half aggregation's start=True
    wipes those banks.
    """
    nc = tc.nc
    mult = mybir.AluOpType.mult
    subop = mybir.AluOpType.subtract
    IHW = JC * 256                # bytes per i-half in at8's free dim
    NU = 16                       # units: 8 A-half + 8 B-half jc-quads

    with (
        tc.tile_pool(name="big", bufs=1) as big,
        tc.tile_pool(name="small", bufs=1) as small,
        tc.tile_pool(name="mtp", bufs=6) as mtp,
        tc.tile_pool(name="osb", bufs=4) as osb,
        tc.tile_pool(name="psc", bufs=2, space="PSUM") as psc,
        tc.tile_pool(name="pso", bufs=1, space="PSUM") as pso,
    ):
        at8 = big.tile([128, 2 * IHW], F8, tag="at8")       # [p, ih, jc, 256]
        hh8 = big.tile([128, JC * D], F8, tag="hh8")        # h hi [p, jc, d]
        hl8 = big.tile([128, JC * D], F8, tag="hl8")        # h lo [p, jc, d]
        wt = small.tile([3, N], F16, tag="wt")              # W'.T
        w4 = small.tile([128, JC * 4], F16, tag="w4")       # W'|ones (j part)
        id16 = small.tile([128, 128], F16, tag="id16")
        scr = small.tile([128, 512], F16, tag="scr")        # warm-up source
        rN16 = small.tile([128, IC * H], F16, tag="rN16")   # 1/denom'
        rs32 = small.tile([128, IC], F32, tag="rs32")       # rowsum per ic
        rT16 = small.tile([3, SH], F16, tag="rT16")         # R'.T [k, i]

        psO = [
            pso.tile([128, D], F32, tag=f"psO{ic}", name=f"psO{ic}")
            for ic in range(IC)
        ]
        # R-chain scratch aliased into psO[2]/psO[3] (consumed before the
        # first B-half agg start wipes those banks)
        psD2 = psO[2][:, 0:16]                              # [128, 16] f32
        psRT = psO[3][0:3, 0:256].bitcast(F16)              # [3, 512] f16

        # ---------------- loads ----------------
        # sync/HWDGE queue: at8 A-half pieces (A denominators stream with
        # them), id/wt, first h-piece, at8 B-half pieces, remaining h pieces.
        # w4 on the scalar queue (needed by the first denominator matmuls).
        nc.scalar.dma_start(out=w4[:], in_=w4_in)
        AP_PIECES = [12, 12, 7, 1]

        a8_r = a8_in.rearrange("p (ih jc x) -> p ih jc x", ih=2, jc=JC)

        def at8_pieces(ih):
            off = 0
            bnds = []
            av = at8[:].rearrange("p (ih jc x) -> p ih jc x", ih=2, jc=JC)
            for n_ in AP_PIECES:
                nc.sync.dma_start(out=av[:, ih, off:off + n_],
                                  in_=a8_r[:, ih, off:off + n_])
                bnds.append((off, off + n_))
                off += n_
            return bnds

        bounds_a = at8_pieces(0)
        nc.sync.dma_start(out=id16[:], in_=id_in)
        nc.sync.dma_start(out=wt[:], in_=wt_in)
        hh_r = hh_in.rearrange("p (jc d) -> p jc d", d=D)
        hl_r = hl_in.rearrange("p (jc d) -> p jc d", d=D)
        hh_v = hh8[:].rearrange("p (jc d) -> p jc d", d=D)
        hl_v = hl8[:].rearrange("p (jc d) -> p jc d", d=D)

        def h_piece(j0, j1):
            nc.sync.dma_start(out=hh_v[:, j0:j1], in_=hh_r[:, j0:j1])
            nc.sync.dma_start(out=hl_v[:, j0:j1], in_=hl_r[:, j0:j1])

        h_piece(0, 4)
        bounds_b = at8_pieces(1)
        h_piece(4, 8)
        h_piece(8, 16)
        h_piece(16, 24)
        h_piece(24, 32)

        nc.vector.memset(scr[:], 0.0)
        # warm the ACT table (LoadActFuncSet) off the critical path
        actw = small.tile([1, 2], F16, tag="actw")
        nc.scalar.copy(actw[:], scr[0:1, 0:2])

        n_warm = [0]

        def warm(n_):
            # warm-up targets rotate over psO[0]/psO[1]: their garbage is
            # wiped by the first real agg matmul's start=True
            for _ in range(n_):
                nc.tensor.matmul(
                    psO[n_warm[0] % 2][:], scr[:, 0:128], scr[:],
                    start=True, stop=True, skip_group_check=True,
                )
                n_warm[0] += 1

        at8_v = at8[:].rearrange("p (ih jc x) -> p ih jc x", ih=2, jc=JC)
        den_state = {"first": True}

        def denoms(ih, j0, j1, last):
            # psD2[p_i, ic*4+k] += sum_j A[i,j] W'[j,k]; k=3 gives rowsum.
            # Single accumulation super-group across BOTH halves: start only
            # on the very first matmul (pending-zero covers the bank).
            for jc in range(j0, j1):
                for ii in range(2):
                    ic = ih * 2 + ii
                    nc.tensor.matmul(
                        psD2[:, ic * 4:(ic + 1) * 4],
                        at8_v[:, ih, jc, ii * 128:(ii + 1) * 128],
                        w4[:, jc * 4:(jc + 1) * 4],
                        start=den_state["first"],
                        stop=last and (jc == j1 - 1 and ii == 1),
                        skip_group_check=True,
                    )
                    den_state["first"] = False

        def r_chain(ih):
            # R' = 1/denom' (fp16) for this half, transposed to [k, i].
            # rowsum is folded into the psO stores later.
            psD2_v = psD2.rearrange("p (ic s) -> p ic s", s=4)
            with nc.allow_low_precision(reason="R' fits fp16"):
                nc.vector.reciprocal(
                    rN16[:, ih * 2 * H:(ih + 1) * 2 * H].rearrange(
                        "p (ic k) -> p ic k", k=H),
                    psD2_v[:, 2 * ih:2 * ih + 2, 0:H],
                )
            nc.vector.tensor_copy(
                rs32[:, 2 * ih:2 * ih + 2], psD2_v[:, 2 * ih:2 * ih + 2, 3]
            )
            for ii in range(2):
                ic = ih * 2 + ii
                nc.tensor.transpose(
                    psRT[:, ic * 128:(ic + 1) * 128],
                    rN16[:, ic * H:(ic + 1) * H],
                    id16[:],
                )
            nc.vector.tensor_copy(
                rT16[:, ih * 256:(ih + 1) * 256],
                psRT[:, ih * 256:(ih + 1) * 256],
            )

        # ---------------- sweep units ----------------
        # unit u = (ih, g): jc-quad 4g..4g+3 x i-half ih; pairs (2g, 2g+1)
        UNITS = [(0, 0), (0, 1), (0, 2), (0, 3), (0, 4),
                 (1, 0), (0, 5), (1, 1), (0, 6), (1, 2), (0, 7),
                 (1, 3), (1, 4), (1, 5), (1, 6), (1, 7)]
        cp_eng = ["act"] * NU
        cp_eng[0] = "dve"
        sub_eng = [
            "dve" if (u == 0 or u >= NU - 2) else "pool"
            for u in range(NU)
        ]
        hh8_v = hh8[:].rearrange("p (pr two d) -> p pr two d", two=2, d=D)
        hl8_v = hl8[:].rearrange("p (pr two d) -> p pr two d", two=2, d=D)

        his = {}
        los = {}

        def front(u):
            ih, g = UNITS[u]
            ctp = psc.tile([128, 4 * 256], F32, tag="ctp", name=f"ctp{u}")
            for q in range(4):
                jc = 4 * g + q
                nc.tensor.matmul(
                    ctp[:, q * 256:(q + 1) * 256],
                    wt[0:3, jc * 128:(jc + 1) * 128],
                    rT16[:, ih * 256:(ih + 1) * 256],
                    start=True, stop=True,
                    tile_position=(0, 0),
                )
            mt16 = mtp.tile([128, 1024], F16, tag="mt16", name=f"mt16_{u}")
            nc.vector.tensor_tensor(
                mt16[:], at8_v[:, ih, 4 * g:4 * g + 4], ctp[:], op=mult,
            )
            hi8 = mtp.tile([128, 1024], F8, tag="hi8", name=f"hi8_{u}")
            lo8 = mtp.tile([128, 1024], F8, tag="lo8", name=f"lo8_{u}")
            if cp_eng[u] == "act":
                nc.scalar.copy(hi8[:], mt16[:])
            elif cp_eng[u] == "dve":
                nc.vector.tensor_copy(hi8[:], mt16[:])
            else:
                nc.gpsimd.tensor_copy(hi8[:], mt16[:])
            if sub_eng[u] == "dve":
                nc.vector.tensor_tensor(lo8[:], mt16[:], hi8[:], op=subop)
            else:
                nc.gpsimd.tensor_tensor(lo8[:], mt16[:], hi8[:], op=subop)
            his[u], los[u] = hi8, lo8

        # grouped stores: two ic per out-DMA; rowsum folds in as the scale.
        # Group 0 (ic0/1, A-half) completes mid-program and is fully hidden.
        ot2 = [
            osb.tile([128, 2 * D], F16, tag=f"ot{g}", name=f"ot{g}")
            for g in range(2)
        ]

        def store(ic):
            g, half = divmod(ic, 2)
            dst = ot2[g][:, half * D:(half + 1) * D]
            if ic % 2 == 0:
                nc.scalar.mul(dst, psO[ic][:], rs32[:, ic:ic + 1])
            else:
                nc.vector.tensor_scalar(
                    dst, psO[ic][:], rs32[:, ic:ic + 1], None, op0=mult
                )
            if half == 1:
                out_g = out[g * 256:(g + 1) * 256, :].rearrange(
                    "(two p) d -> p two d", two=2
                )
                (nc.sync if g == 0 else nc.scalar).dma_start(
                    out=out_g,
                    in_=ot2[g][:].rearrange("p (two d) -> p two d", two=2),
                )

        TERMS = ((0, 0), (0, 1), (1, 0))  # (hi/lo, hh/hl)
        started = set()
        last_u = {}                        # ih -> last unit index
        for u, (ih, g) in enumerate(UNITS):
            last_u[ih] = u

        def agg(u):
            ih, g = UNITS[u]
            hi8_v = his[u][:].rearrange("p (q x) -> p q x", q=4)
            lo8_v = los[u][:].rearrange("p (q x) -> p q x", q=4)
            lts = (hi8_v, lo8_v)
            rts = (hh8_v, hl8_v)
            final = last_u[ih] == u
            if final:
                order = [(ii, pr, t) for ii in range(2) for pr in range(2)
                         for t in range(3)]
            else:
                order = [(ii, pr, t) for t in range(3) for pr in range(2)
                         for ii in range(2)]
            for ii, pr, t in order:
                ic = 2 * ih + ii
                lt, rt = lts[TERMS[t][0]], rts[TERMS[t][1]]
                st = ic not in started
                started.add(ic)
                # lhsT: DR planes = the two jc of pair pr, i-slice ii
                lv = lt[:, 2 * pr:2 * pr + 2, ii * 128:(ii + 1) * 128]
                nc.tensor.matmul(
                    psO[ic][:],
                    lv,
                    rt[:, 2 * g + pr],
                    start=st,
                    stop=final and (pr == 1 and t == 2),
                    perf_mode=DR,
                    skip_group_check=True,
                )
                if final and pr == 1 and t == 2:
                    store(ic)

        # ---------------- emission schedule ----------------
        # PE in-order stream: warms + A-denominators stream with the A
        # pieces; A-half R chain; then the software-pipelined units with the
        # B denominators + B R chain sprinkled between early units so they
        # execute as the B pieces land without blocking A aggregations.
        warm(4)
        for pi, (j0, j1) in enumerate(bounds_a):
            denoms(0, j0, j1, False)
            if pi < len(bounds_a) - 2:
                warm(3)
        r_chain(0)

        LAG = 3
        emitted_b = [0]

        def maybe_emit_b(slot):
            # slot: how many units have been fronted so far
            if slot == 3 and emitted_b[0] == 0:
                denoms(1, *bounds_b[0], False)
                denoms(1, *bounds_b[1], False)
                emitted_b[0] = 2
            elif slot == 4 and emitted_b[0] == 2:
                denoms(1, *bounds_b[2], False)
                denoms(1, *bounds_b[3], True)
                r_chain(1)
                emitted_b[0] = 4

        for u in range(NU + LAG):
            if u < NU:
                front(u)
            maybe_emit_b(u + 1)
            if u >= LAG:
                agg(u - LAG)


_CACHE = {}


def _build1():
    if "p1" in _CACHE:
        return _CACHE["p1"]
    nc = bacc.Bacc("TRN2", target_bir_lowering=False, debug=False,
                   num_devices=NCORES)
    hst_in = nc.dram_tensor("hst_in", [128, IC * DC * 128], F16,
                            kind="ExternalInput").ap()
    p_in = nc.dram_tensor("p_in", [128, DC * H], F16, kind="ExternalInput").ap()
    w_out = nc.dram_tensor("w_out", [128, IC * H], F16,
                           kind="ExternalOutput").ap()
    with tile.TileContext(nc) as tc:
        _body1(tc, hst_in, p_in, w_out)
    nc.compile()
    _CACHE["p1"] = nc
    return nc


def _build2():
    if "p2" in _CACHE:
        return _CACHE["p2"]
    nc = bacc.Bacc("TRN2", target_bir_lowering=False, debug=False,
                   num_devices=NCORES)
    a8_in = nc.dram_tensor("a8_in", [128, JC * SH], F8,
                           kind="ExternalInput").ap()
    hh_in = nc.dram_tensor("hh_in", [128, JC * D], F8,
                           kind="ExternalInput").ap()
    hl_in = nc.dram_tensor("hl_in", [128, JC * D], F8,
                           kind="ExternalInput").ap()
    wt_in = nc.dram_tensor("wt_in", [3, N], F16, kind="ExternalInput").ap()
    w4_in = nc.dram_tensor("w4_in", [128, JC * 4], F16,
                           kind="ExternalInput").ap()
    id_in = nc.dram_tensor("id_in", [128, 128], F16, kind="ExternalInput").ap()
    out = nc.dram_tensor("out", [SH, D], F16, kind="ExternalOutput").ap()
    with tile.TileContext(nc) as tc:
        _body2(tc, a8_in, hh_in, hl_in, wt_in, w4_in, id_in, out)
    nc.compile()
    _CACHE["p2"] = nc
    return nc


def kernel(graph_info, h, P, _trace=False, _results_out=None):
    graph_info = np.ascontiguousarray(graph_info, dtype=np.float32)
    h = np.ascontiguousarray(h, dtype=np.float32)
    P = np.ascontiguousarray(P, dtype=np.float32)
    nc1 = _build1()
    nc2 = _build2()

    # host-side shard/layout prep (pure data movement + dtype casts)
    h16_full = h.astype(np.float16)
    p16_host = np.ascontiguousarray(
        P.astype(np.float16).reshape(DC, 128, H).transpose(1, 0, 2)
    ).reshape(128, DC * H)
    in1 = []
    for c in range(NCORES):
        hsT = h16_full[c * SH:(c + 1) * SH, :].T  # [D, SH]
        hst_host = np.ascontiguousarray(
            hsT.reshape(DC, 128, IC, 128).transpose(1, 2, 0, 3)
        ).reshape(128, IC * DC * 128)
        in1.append({"hst_in": hst_host, "p_in": p16_host})
    res1 = bass_utils.run_bass_kernel_spmd(
        nc1, in1, core_ids=list(range(NCORES)), trace=_trace
    )
    w_full = np.concatenate(
        [
            res1.results[c]["w_out"]
            .reshape(128, IC, H).transpose(1, 0, 2).reshape(SH, H)
            for c in range(NCORES)
        ],
        axis=0,
    )  # [N, 3] fp16, scaled by 2^-4

    wt_host = np.ascontiguousarray(w_full.T)  # [3, N]
    w4_host = np.ascontiguousarray(
        np.concatenate(
            [w_full.reshape(JC, 128, H).transpose(1, 0, 2),
             np.ones((128, JC, 1), np.float16)],
            axis=2,
        ).reshape(128, JC * 4)
    )
    id_host = np.eye(128, dtype=np.float16)

    # fp8 hi/lo split of h (host-side re-encoding; h = hh + hl up to e4m3^2)
    h_hi = np.clip(h, -240, 240).astype(NP_F8)
    h_lo = (h - h_hi.astype(np.float32)).astype(NP_F8)
    hh_host = np.ascontiguousarray(
        h_hi.reshape(JC, 128, D).transpose(1, 0, 2)).reshape(128, JC * D)
    hl_host = np.ascontiguousarray(
        h_lo.reshape(JC, 128, D).transpose(1, 0, 2)).reshape(128, JC * D)

    in2 = []
    for c in range(NCORES):
        at = np.ascontiguousarray(
            graph_info[c * SH:(c + 1) * SH, :].T
        ).astype(NP_F8)                      # [N(j), SH(i)]
        # pack [p, ihalf, jc, 256]: i-half-major so the A-half is contiguous
        a8_host = np.ascontiguousarray(
            at.reshape(JC, 128, 2, 256).transpose(1, 2, 0, 3)
        ).reshape(128, JC * SH)
        in2.append({
            "a8_in": a8_host,
            "hh_in": hh_host,
            "hl_in": hl_host,
            "wt_in": wt_host,
            "w4_in": w4_host,
            "id_in": id_host,
        })
    res2 = bass_utils.run_bass_kernel_spmd(
        nc2, in2, core_ids=list(range(NCORES)), trace=_trace
    )
    if _results_out is not None:
        _results_out.extend([res1, res2])
    return np.concatenate(
        [res2.results[c]["out"].astype(np.float32) for c in range(NCORES)],
        axis=0,
    )


# revision 37
# speedup vs baseline: 1.0444x; 1.0444x over previous
"""GAT-style attention (gnn_message_passing) Trainium2 kernel, 8-core row-parallel.

Math (algebraically identical to the reference masked-softmax attention):
  E = relu(h @ P)                 [N,3]
  W' = max(exp(E - 4ln2), 1/16)   (= exp(relu(E))/16, fp16-safe range)
  denom'[i,k] = sum_j A[i,j] W'[j,k]   (k=3 slot sums ones -> rowsum[i])
  R'[i,k] = rowsum[i] / denom'[i,k]
  ct[j,i]  = sum_k W'[j,k] R'[i,k] = rowsum[i] * C[i,j]
  mt[j,i]  = A[i,j] * ct[j,i]
  out[i,:] = sum_j mt[j,i] h[j,:]

Two SPMD programs (cost-modeled collectives are ~15us fixed -> too slow; the
tiny [4096,3] W matrix crosses cores via a host gather between programs):
  P1 (per core): W'-shard [512,3] from host-transposed h-shard (fp16 — fp8
      h.T fails the error budget through the exponential).
  host: concat W'-shards; build wt [3,N] / w4 (W'|ones) layouts; cast
      A-shard.T to fp8 (binary, exact); split h into fp8 hi/lo halves
      (h = h_hi + h_lo, each e4m3; lossless-ish re-encoding).
  P2 (per core): denominators stream with the A.T pieces (at8-stationary
      matmuls, one PSUM accumulation "super-group"), R' chain, then 16
      jc-pair sweeps:
        ct pair (fp16 matmuls, [128,1024] PSUM)
        mt16 = at8 * ct          (DVE, the only full-size PSUM touch)
        mt_hi8 = fp8(mt16)       (ACT copies, a couple on POOL for balance)
        mt_lo8 = mt16 - mt_hi8   (DVE/POOL split)
        psO[ic] += DoubleRow fp8 matmuls: mt_hi.T@h_hi + mt_hi.T@h_lo
                   + mt_lo.T@h_hi   (3-term split => 0.3% rel err, 4x
                   cheaper than fp16 per the 0.5 cycles/row DR rate)
      Warm-up matmuls during the initial load defeat the PE clock ramp.
"""

import numpy as np
import ml_dtypes

import concourse.bass as bass
import concourse.mybir as mybir
import concourse.tile as tile
from concourse import bacc
from concourse import bass_utils

N = 4096
D = 512
H = 3
NCORES = 8
SH = N // NCORES          # 512 output rows per core
JC = N // 128             # 32 j-chunks
IC = SH // 128            # 4 i-chunks
DC = D // 128             # 4 d-chunks
NP = JC // 2              # 16 jc-pairs
F8 = mybir.dt.float8e4
F16 = mybir.dt.float16
F32 = mybir.dt.float32
LN2x4 = float(4.0 * np.log(2.0))   # W scaled by 2^-4 to stay in fp16 range
NP_F8 = ml_dtypes.float8_e4m3
DR = mybir.MatmulPerfMode.DoubleRow


def _body1(tc, hst_in, p_in, w_out):
    """P1: W'-shard [SH,3] from hst [128, IC*DC*128] (h-shard.T, jc-major:
    hst[:, jc, dc, :] = h.T d-chunk dc for j-chunk jc). Loaded and computed
    in per-jc waves so each wave's exp/max/w_out DMA overlaps the next
    wave's hst transfer + completion semaphore."""
    nc = tc.nc
    with (
        tc.tile_pool(name="sb1", bufs=1) as sb,
        tc.tile_pool(name="ps1", bufs=1, space="PSUM") as ps,
    ):
        hst = sb.tile([128, IC * DC * 128], F16, tag="hst")
        p16 = sb.tile([128, DC * H], F16, tag="p16")
        wsE = sb.tile([128, IC * H], F16, tag="wsE")
        ebias = sb.tile([128, 1], F32, tag="ebias")
        nc.gpsimd.dma_start(out=p16[:], in_=p_in)
        nc.vector.memset(ebias[:], -LN2x4)
        hst_v = hst[:].rearrange("p (g x) -> g p x", g=2)
        hin_v = hst_in.rearrange("p (g x) -> g p x", g=2)
        for g in range(2):
            nc.sync.dma_start(out=hst_v[g], in_=hin_v[g])

        # one PSUM tile spanning 4 banks: E group per jc, single exp at the end
        psE = ps.tile([128, IC * 512], F32, tag="psE", name="psE")
        for jc in range(IC):
            for dc in range(DC):
                nc.tensor.matmul(
                    psE[:, jc * 512: jc * 512 + H],
                    hst[:, (jc * DC + dc) * 128: (jc * DC + dc + 1) * 128],
                    p16[:, dc * H:(dc + 1) * H],
                    start=(dc == 0),
                    stop=(dc == DC - 1),
                )
        nc.scalar.activation(
            wsE[:].rearrange("p (jc k) -> p jc k", k=H),
            psE[:].rearrange("p (jc x) -> p jc x", x=512)[:, :, 0:H],
            mybir.ActivationFunctionType.Exp,
            bias=ebias[:], scale=1.0,
        )
        nc.vector.tensor_scalar_max(wsE[:], wsE[:], 0.0625)
        nc.sync.dma_start(out=w_out, in_=wsE[:])


def _body2(tc, a8_in, hh_in, hl_in, wt_in, w4_in, id_in, out):
    """P2: denominators + R' chain + 16 uniform sweep units with 3-term fp8
    DoubleRow aggregation.

    a8_in is A-shard.T fp8 packed [p, ihalf, jc, 256]: the i (output-row)
    space is split into two halves of 256. A unit = (ihalf, jc-quad): a
    [128, 1024] slab covering 4 j-chunks x one i-half = 2 DR pair-planes x
    2 ic blocks. A-half units depend only on the A-half denominators (the
    first 1 MB of at8), so the sweep starts ~3us earlier than a full-R
    schedule; B-half units start once the full at8 has landed.

    PSUM budget trick: the denominator accumulator psD2 and the R-transpose
    scratch psRT live inside psO[2]/psO[3]'s banks (bitcast slices). They
    are fully consumed before the first B-half aggregation's start=True
    wipes those banks.
    """
    nc = tc.nc
    mult = mybir.AluOpType.mult
    subop = mybir.AluOpType.subtract
    IHW = JC * 256                # bytes per i-half in at8's free dim
    NU = 16                       # units: 8 A-half + 8 B-half jc-quads

    with (
        tc.tile_pool(name="big", bufs=1) as big,
        tc.tile_pool(name="small", bufs=1) as small,
        tc.tile_pool(name="mtp", bufs=6) as mtp,
        tc.tile_pool(name="osb", bufs=4) as osb,
        tc.tile_pool(name="psc", bufs=2, space="PSUM") as psc,
        tc.tile_pool(name="pso", bufs=1, space="PSUM") as pso,
    ):
        at8 = big.tile([128, 2 * IHW], F8, tag="at8")       # [p, ih, jc, 256]
        hh8 = big.tile([128, JC * D], F8, tag="hh8")        # h hi [p, jc, d]
        hl8 = big.tile([128, JC * D], F8, tag="hl8")        # h lo [p, jc, d]
        wt = small.tile([3, N], F16, tag="wt")              # W'.T
        w4 = small.tile([128, JC * 4], F16, tag="w4")       # W'|ones (j part)
        id16 = small.tile([128, 128], F16, tag="id16")
        scr = small.tile([128, 512], F16, tag="scr")        # warm-up source
        rN16 = small.tile([128, IC * H], F16, tag="rN16")   # 1/denom'
        rs32 = small.tile([128, IC], F32, tag="rs32")       # rowsum per ic
        rT16 = small.tile([3, SH], F16, tag="rT16")         # R'.T [k, i]

        psO = [
            pso.tile([128, D], F32, tag=f"psO{ic}", name=f"psO{ic}")
            for ic in range(IC)
        ]
        # R-chain scratch aliased into psO[2]/psO[3] (consumed before the
        # first B-half agg start wipes those banks)
        psD2 = psO[2][:, 0:16]                              # [128, 16] f32
        psRT = psO[3][0:3, 0:256].bitcast(F16)              # [3, 512] f16

        # ---------------- loads ----------------
        # sync/HWDGE queue: at8 A-half pieces (A denominators stream with
        # them), id/wt, first h-piece, at8 B-half pieces, remaining h pieces.
        # w4 on the scalar queue (needed by the first denominator matmuls).
        nc.scalar.dma_start(out=w4[:], in_=w4_in)
        AP_PIECES = [12, 12, 7, 1]

        a8_r = a8_in.rearrange("p (ih jc x) -> p ih jc x", ih=2, jc=JC)

        def at8_pieces(ih):
            off = 0
            bnds = []
            av = at8[:].rearrange("p (ih jc x) -> p ih jc x", ih=2, jc=JC)
            for n_ in AP_PIECES:
                nc.sync.dma_start(out=av[:, ih, off:off + n_],
                                  in_=a8_r[:, ih, off:off + n_])
                bnds.append((off, off + n_))
                off += n_
            return bnds

        bounds_a = at8_pieces(0)
        nc.sync.dma_start(out=id16[:], in_=id_in)
        nc.sync.dma_start(out=wt[:], in_=wt_in)
        hh_r = hh_in.rearrange("p (jc d) -> p jc d", d=D)
        hl_r = hl_in.rearrange("p (jc d) -> p jc d", d=D)
        hh_v = hh8[:].rearrange("p (jc d) -> p jc d", d=D)
        hl_v = hl8[:].rearrange("p (jc d) -> p jc d", d=D)

        def h_piece(j0, j1):
            nc.sync.dma_start(out=hh_v[:, j0:j1], in_=hh_r[:, j0:j1])
            nc.sync.dma_start(out=hl_v[:, j0:j1], in_=hl_r[:, j0:j1])

        h_piece(0, 4)
        bounds_b = at8_pieces(1)
        h_piece(4, 8)
        h_piece(8, 16)
        h_piece(16, 24)
        h_piece(24, 32)

        nc.vector.memset(scr[:], 0.0)
        # warm the ACT table (LoadActFuncSet) off the critical path
        actw = small.tile([1, 2], F16, tag="actw")
        nc.scalar.copy(actw[:], scr[0:1, 0:2])

        n_warm = [0]

        def warm(n_):
            # warm-up targets rotate over psO[0]/psO[1]: their garbage is
            # wiped by the first real agg matmul's start=True
            for _ in range(n_):
                nc.tensor.matmul(
                    psO[n_warm[0] % 2][:], scr[:, 0:128], scr[:],
                    start=True, stop=True, skip_group_check=True,
                )
                n_warm[0] += 1

        at8_v = at8[:].rearrange("p (ih jc x) -> p ih jc x", ih=2, jc=JC)
        den_state = {"first": True}

        def denoms(ih, j0, j1, last):
            # psD2[p_i, ic*4+k] += sum_j A[i,j] W'[j,k]; k=3 gives rowsum.
            # Single accumulation super-group across BOTH halves: start only
            # on the very first matmul (pending-zero covers the bank).
            for jc in range(j0, j1):
                for ii in range(2):
                    ic = ih * 2 + ii
                    nc.tensor.matmul(
                        psD2[:, ic * 4:(ic + 1) * 4],
                        at8_v[:, ih, jc, ii * 128:(ii + 1) * 128],
                        w4[:, jc * 4:(jc + 1) * 4],
                        start=den_state["first"],
                        stop=last and (jc == j1 - 1 and ii == 1),
                        skip_group_check=True,
                    )
                    den_state["first"] = False

        def r_chain(ih):
            # R' = 1/denom' (fp16) for this half, transposed to [k, i].
            # rowsum is folded into the psO stores later.
            psD2_v = psD2.rearrange("p (ic s) -> p ic s", s=4)
            with nc.allow_low_precision(reason="R' fits fp16"):
                nc.vector.reciprocal(
                    rN16[:, ih * 2 * H:(ih + 1) * 2 * H].rearrange(
                        "p (ic k) -> p ic k", k=H),
                    psD2_v[:, 2 * ih:2 * ih + 2, 0:H],
                )
            nc.vector.tensor_copy(
                rs32[:, 2 * ih:2 * ih + 2], psD2_v[:, 2 * ih:2 * ih + 2, 3]
            )
            for ii in range(2):
                ic = ih * 2 + ii
                nc.tensor.transpose(
                    psRT[:, ic * 128:(ic + 1) * 128],
                    rN16[:, ic * H:(ic + 1) * H],
                    id16[:],
                )
            nc.vector.tensor_copy(
                rT16[:, ih * 256:(ih + 1) * 256],
                psRT[:, ih * 256:(ih + 1) * 256],
            )

        # ---------------- sweep units ----------------
        # unit u = (ih, g): jc-quad 4g..4g+3 x i-half ih; pairs (2g, 2g+1)
        UNITS = [(0, 0), (0, 1), (0, 2), (0, 3), (0, 4),
                 (1, 0), (0, 5), (1, 1), (0, 6), (1, 2), (0, 7),
                 (1, 3), (1, 4), (1, 5), (1, 6), (1, 7)]
        cp_eng = ["act"] * NU
        cp_eng[0] = "dve"
        sub_eng = [
            "dve" if (u % 5 == 0 or u >= NU - 3) else "pool"
            for u in range(NU)
        ]
        hh8_v = hh8[:].rearrange("p (pr two d) -> p pr two d", two=2, d=D)
        hl8_v = hl8[:].rearrange("p (pr two d) -> p pr two d", two=2, d=D)

        his = {}
        los = {}

        def front(u):
            ih, g = UNITS[u]
            ctp = psc.tile([128, 4 * 256], F32, tag="ctp", name=f"ctp{u}")
            for q in range(4):
                jc = 4 * g + q
                nc.tensor.matmul(
                    ctp[:, q * 256:(q + 1) * 256],
                    wt[0:3, jc * 128:(jc + 1) * 128],
                    rT16[:, ih * 256:(ih + 1) * 256],
                    start=True, stop=True,
                    tile_position=(0, 0),
                )
            mt16 = mtp.tile([128, 1024], F16, tag="mt16", name=f"mt16_{u}")
            nc.vector.tensor_tensor(
                mt16[:], at8_v[:, ih, 4 * g:4 * g + 4], ctp[:], op=mult,
            )
            hi8 = mtp.tile([128, 1024], F8, tag="hi8", name=f"hi8_{u}")
            lo8 = mtp.tile([128, 1024], F8, tag="lo8", name=f"lo8_{u}")
            if cp_eng[u] == "act":
                nc.scalar.copy(hi8[:], mt16[:])
            elif cp_eng[u] == "dve":
                nc.vector.tensor_copy(hi8[:], mt16[:])
            else:
                nc.gpsimd.tensor_copy(hi8[:], mt16[:])
            if sub_eng[u] == "dve":
                nc.vector.tensor_tensor(lo8[:], mt16[:], hi8[:], op=subop)
            else:
                nc.gpsimd.tensor_tensor(lo8[:], mt16[:], hi8[:], op=subop)
            his[u], los[u] = hi8, lo8

        # grouped stores: two ic per out-DMA; rowsum folds in as the scale.
        # Group 0 (ic0/1, A-half) completes mid-program and is fully hidden.
        ot2 = [
            osb.tile([128, 2 * D], F16, tag=f"ot{g}", name=f"ot{g}")
            for g in range(2)
        ]

        def store(ic):
            g, half = divmod(ic, 2)
            dst = ot2[g][:, half * D:(half + 1) * D]
            if ic % 2 == 0:
                nc.scalar.mul(dst, psO[ic][:], rs32[:, ic:ic + 1])
            else:
                nc.vector.tensor_scalar(
                    dst, psO[ic][:], rs32[:, ic:ic + 1], None, op0=mult
                )
            if half == 1:
                out_g = out[g * 256:(g + 1) * 256, :].rearrange(
                    "(two p) d -> p two d", two=2
                )
                (nc.sync if g == 0 else nc.scalar).dma_start(
                    out=out_g,
                    in_=ot2[g][:].rearrange("p (two d) -> p two d", two=2),
                )

        TERMS = ((0, 0), (0, 1), (1, 0))  # (hi/lo, hh/hl)
        started = set()
        last_u = {}                        # ih -> last unit index
        for u, (ih, g) in enumerate(UNITS):
            last_u[ih] = u

        def agg(u):
            ih, g = UNITS[u]
            hi8_v = his[u][:].rearrange("p (q x) -> p q x", q=4)
            lo8_v = los[u][:].rearrange("p (q x) -> p q x", q=4)
            lts = (hi8_v, lo8_v)
            rts = (hh8_v, hl8_v)
            final = last_u[ih] == u
            if final:
                order = [(ii, pr, t) for ii in range(2) for pr in range(2)
                         for t in range(3)]
            else:
                order = [(ii, pr, t) for t in range(3) for pr in range(2)
                         for ii in range(2)]
            for ii, pr, t in order:
                ic = 2 * ih + ii
                lt, rt = lts[TERMS[t][0]], rts[TERMS[t][1]]
                st = ic not in started
                started.add(ic)
                # lhsT: DR planes = the two jc of pair pr, i-slice ii
                lv = lt[:, 2 * pr:2 * pr + 2, ii * 128:(ii + 1) * 128]
                nc.tensor.matmul(
                    psO[ic][:],
                    lv,
                    rt[:, 2 * g + pr],
                    start=st,
                    stop=final and (pr == 1 and t == 2),
                    perf_mode=DR,
                    skip_group_check=True,
                )
                if final and pr == 1 and t == 2:
                    store(ic)

        # ---------------- emission schedule ----------------
        # PE in-order stream: warms + A-denominators stream with the A
        # pieces; A-half R chain; then the software-pipelined units with the
        # B denominators + B R chain sprinkled between early units so they
        # execute as the B pieces land without blocking A aggregations.
        warm(4)
        for pi, (j0, j1) in enumerate(bounds_a):
            denoms(0, j0, j1, False)
            if pi < len(bounds_a) - 2:
                warm(3)
        r_chain(0)

        LAG = 3
        emitted_b = [0]

        def maybe_emit_b(slot):
            # slot: how many units have been fronted so far
            if slot == 4 and emitted_b[0] == 0:
                denoms(1, *bounds_b[0], False)
                denoms(1, *bounds_b[1], False)
                emitted_b[0] = 2
            elif slot == 5 and emitted_b[0] == 2:
                denoms(1, *bounds_b[2], False)
                denoms(1, *bounds_b[3], True)
                r_chain(1)
                emitted_b[0] = 4

        for u in range(NU + LAG):
            if u < NU:
                front(u)
            maybe_emit_b(u + 1)
            if u >= LAG:
                agg(u - LAG)


_CACHE = {}


def _build1():
    if "p1" in _CACHE:
        return _CACHE["p1"]
    nc = bacc.Bacc("TRN2", target_bir_lowering=False, debug=False,
                   num_devices=NCORES)
    hst_in = nc.dram_tensor("hst_in", [128, IC * DC * 128], F16,
                            kind="ExternalInput").ap()
    p_in = nc.dram_tensor("p_in", [128, DC * H], F16, kind="ExternalInput").ap()
    w_out = nc.dram_tensor("w_out", [128, IC * H], F16,
                           kind="ExternalOutput").ap()
    with tile.TileContext(nc) as tc:
        _body1(tc, hst_in, p_in, w_out)
    nc.compile()
    _CACHE["p1"] = nc
    return nc


def _build2():
    if "p2" in _CACHE:
        return _CACHE["p2"]
    nc = bacc.Bacc("TRN2", target_bir_lowering=False, debug=False,
                   num_devices=NCORES)
    a8_in = nc.dram_tensor("a8_in", [128, JC * SH], F8,
                           kind="ExternalInput").ap()
    hh_in = nc.dram_tensor("hh_in", [128, JC * D], F8,
                           kind="ExternalInput").ap()
    hl_in = nc.dram_tensor("hl_in", [128, JC * D], F8,
                           kind="ExternalInput").ap()
    wt_in = nc.dram_tensor("wt_in", [3, N], F16, kind="ExternalInput").ap()
    w4_in = nc.dram_tensor("w4_in", [128, JC * 4], F16,
                           kind="ExternalInput").ap()
    id_in = nc.dram_tensor("id_in", [128, 128], F16, kind="ExternalInput").ap()
    out = nc.dram_tensor("out", [SH, D], F16, kind="ExternalOutput").ap()
    with tile.TileContext(nc) as tc:
        _body2(tc, a8_in, hh_in, hl_in, wt_in, w4_in, id_in, out)
    nc.compile()
    _CACHE["p2"] = nc
    return nc


def kernel(graph_info, h, P, _trace=False, _results_out=None):
    graph_info = np.ascontiguousarray(graph_info, dtype=np.float32)
    h = np.ascontiguousarray(h, dtype=np.float32)
    P = np.ascontiguousarray(P, dtype=np.float32)
    nc1 = _build1()
    nc2 = _build2()

    # host-side shard/layout prep (pure data movement + dtype casts)
    h16_full = h.astype(np.float16)
    p16_host = np.ascontiguousarray(
        P.astype(np.float16).reshape(DC, 128, H).transpose(1, 0, 2)
    ).reshape(128, DC * H)
    in1 = []
    for c in range(NCORES):
        hsT = h16_full[c * SH:(c + 1) * SH, :].T  # [D, SH]
        hst_host = np.ascontiguousarray(
            hsT.reshape(DC, 128, IC, 128).transpose(1, 2, 0, 3)
        ).reshape(128, IC * DC * 128)
        in1.append({"hst_in": hst_host, "p_in": p16_host})
    res1 = bass_utils.run_bass_kernel_spmd(
        nc1, in1, core_ids=list(range(NCORES)), trace=_trace
    )
    w_full = np.concatenate(
        [
            res1.results[c]["w_out"]
            .reshape(128, IC, H).transpose(1, 0, 2).reshape(SH, H)
            for c in range(NCORES)
        ],
        axis=0,
    )  # [N, 3] fp16, scaled by 2^-4

    wt_host = np.ascontiguousarray(w_full.T)  # [3, N]
    w4_host = np.ascontiguousarray(
        np.concatenate(
            [w_full.reshape(JC, 128, H).transpose(1, 0, 2),
             np.ones((128, JC, 1), np.float16)],
            axis=2,
        ).reshape(128, JC * 4)
    )
    id_host = np.eye(128, dtype=np.float16)

    # fp8 hi/lo split of h (host-side re-encoding; h = hh + hl up to e4m3^2)
    h_hi = np.clip(h, -240, 240).astype(NP_F8)
    h_lo = (h - h_hi.astype(np.float32)).astype(NP_F8)
    hh_host = np.ascontiguousarray(
        h_hi.reshape(JC, 128, D).transpose(1, 0, 2)).reshape(128, JC * D)
    hl_host = np.ascontiguousarray(
        h_lo.reshape(JC, 128, D).transpose(1, 0, 2)).reshape(128, JC * D)

    in2 = []
    for c in range(NCORES):
        at = np.ascontiguousarray(
            graph_info[c * SH:(c + 1) * SH, :].T
        ).astype(NP_F8)                      # [N(j), SH(i)]
        # pack [p, ihalf, jc, 256]: i-half-major so the A-half is contiguous
        a8_host = np.ascontiguousarray(
            at.reshape(JC, 128, 2, 256).transpose(1, 2, 0, 3)
        ).reshape(128, JC * SH)
        in2.append({
            "a8_in": a8_host,
            "hh_in": hh_host,
            "hl_in": hl_host,
            "wt_in": wt_host,
            "w4_in": w4_host,
            "id_in": id_host,
        })
    res2 = bass_utils.run_bass_kernel_spmd(
        nc2, in2, core_ids=list(range(NCORES)), trace=_trace
    )
    if _results_out is not None:
        _results_out.extend([res1, res2])
    return np.concatenate(
        [res2.results[c]["out"].astype(np.float32) for c in range(NCORES)],
        axis=0,
    )


# revision 38
# speedup vs baseline: 1.0563x; 1.0114x over previous
"""GAT-style attention (gnn_message_passing) Trainium2 kernel, 8-core row-parallel.

Math (algebraically identical to the reference masked-softmax attention):
  E = relu(h @ P)                 [N,3]
  W' = max(exp(E - 4ln2), 1/16)   (= exp(relu(E))/16, fp16-safe range)
  denom'[i,k] = sum_j A[i,j] W'[j,k]   (k=3 slot sums ones -> rowsum[i])
  R'[i,k] = rowsum[i] / denom'[i,k]
  ct[j,i]  = sum_k W'[j,k] R'[i,k] = rowsum[i] * C[i,j]
  mt[j,i]  = A[i,j] * ct[j,i]
  out[i,:] = sum_j mt[j,i] h[j,:]

Two SPMD programs (cost-modeled collectives are ~15us fixed -> too slow; the
tiny [4096,3] W matrix crosses cores via a host gather between programs):
  P1 (per core): W'-shard [512,3] from host-transposed h-shard (fp16 — fp8
      h.T fails the error budget through the exponential).
  host: concat W'-shards; build wt [3,N] / w4 (W'|ones) layouts; cast
      A-shard.T to fp8 (binary, exact); split h into fp8 hi/lo halves
      (h = h_hi + h_lo, each e4m3; lossless-ish re-encoding).
  P2 (per core): denominators stream with the A.T pieces (at8-stationary
      matmuls, one PSUM accumulation "super-group"), R' chain, then 16
      jc-pair sweeps:
        ct pair (fp16 matmuls, [128,1024] PSUM)
        mt16 = at8 * ct          (DVE, the only full-size PSUM touch)
        mt_hi8 = fp8(mt16)       (ACT copies, a couple on POOL for balance)
        mt_lo8 = mt16 - mt_hi8   (DVE/POOL split)
        psO[ic] += DoubleRow fp8 matmuls: mt_hi.T@h_hi + mt_hi.T@h_lo
                   + mt_lo.T@h_hi   (3-term split => 0.3% rel err, 4x
                   cheaper than fp16 per the 0.5 cycles/row DR rate)
      Warm-up matmuls during the initial load defeat the PE clock ramp.
"""

import numpy as np
import ml_dtypes

import concourse.bass as bass
import concourse.mybir as mybir
import concourse.tile as tile
from concourse import bacc
from concourse import bass_utils

N = 4096
D = 512
H = 3
NCORES = 8
SH = N // NCORES          # 512 output rows per core
JC = N // 128             # 32 j-chunks
IC = SH // 128            # 4 i-chunks
DC = D // 128             # 4 d-chunks
NP = JC // 2              # 16 jc-pairs
F8 = mybir.dt.float8e4
F16 = mybir.dt.float16
F32 = mybir.dt.float32
LN2x4 = float(4.0 * np.log(2.0))   # W scaled by 2^-4 to stay in fp16 range
NP_F8 = ml_dtypes.float8_e4m3
DR = mybir.MatmulPerfMode.DoubleRow


def _body1(tc, hst_in, p_in, w_out):
    """P1: W'-shard [SH,3] from hst [128, IC*DC*128] (h-shard.T, jc-major:
    hst[:, jc, dc, :] = h.T d-chunk dc for j-chunk jc). Loaded and computed
    in per-jc waves so each wave's exp/max/w_out DMA overlaps the next
    wave's hst transfer + completion semaphore."""
    nc = tc.nc
    with (
        tc.tile_pool(name="sb1", bufs=1) as sb,
        tc.tile_pool(name="ps1", bufs=1, space="PSUM") as ps,
    ):
        hst = sb.tile([128, IC * DC * 128], F16, tag="hst")
        p16 = sb.tile([128, DC * H], F16, tag="p16")
        wsE = sb.tile([128, IC * H], F16, tag="wsE")
        ebias = sb.tile([128, 1], F32, tag="ebias")
        nc.gpsimd.dma_start(out=p16[:], in_=p_in)
        nc.vector.memset(ebias[:], -LN2x4)
        hst_v = hst[:].rearrange("p (g x) -> g p x", g=2)
        hin_v = hst_in.rearrange("p (g x) -> g p x", g=2)
        for g in range(2):
            nc.sync.dma_start(out=hst_v[g], in_=hin_v[g])

        # one PSUM tile spanning 4 banks: E group per jc, single exp at the end
        psE = ps.tile([128, IC * 512], F32, tag="psE", name="psE")
        for jc in range(IC):
            for dc in range(DC):
                nc.tensor.matmul(
                    psE[:, jc * 512: jc * 512 + H],
                    hst[:, (jc * DC + dc) * 128: (jc * DC + dc + 1) * 128],
                    p16[:, dc * H:(dc + 1) * H],
                    start=(dc == 0),
                    stop=(dc == DC - 1),
                )
        nc.scalar.activation(
            wsE[:].rearrange("p (jc k) -> p jc k", k=H),
            psE[:].rearrange("p (jc x) -> p jc x", x=512)[:, :, 0:H],
            mybir.ActivationFunctionType.Exp,
            bias=ebias[:], scale=1.0,
        )
        nc.vector.tensor_scalar_max(wsE[:], wsE[:], 0.0625)
        nc.sync.dma_start(out=w_out, in_=wsE[:])


def _body2(tc, a8_in, hh_in, hl_in, wt_in, w4_in, id_in, out):
    """P2: denominators + R' chain + 16 uniform sweep units with 3-term fp8
    DoubleRow aggregation.

    a8_in is A-shard.T fp8 packed [p, ihalf, jc, 256]: the i (output-row)
    space is split into two halves of 256. A unit = (ihalf, jc-quad): a
    [128, 1024] slab covering 4 j-chunks x one i-half = 2 DR pair-planes x
    2 ic blocks. A-half units depend only on the A-half denominators (the
    first 1 MB of at8), so the sweep starts ~3us earlier than a full-R
    schedule; B-half units start once the full at8 has landed.

    PSUM budget trick: the denominator accumulator psD2 and the R-transpose
    scratch psRT live inside psO[2]/psO[3]'s banks (bitcast slices). They
    are fully consumed before the first B-half aggregation's start=True
    wipes those banks.
    """
    nc = tc.nc
    mult = mybir.AluOpType.mult
    subop = mybir.AluOpType.subtract
    IHW = JC * 256                # bytes per i-half in at8's free dim
    NU = 16                       # units: 8 A-half + 8 B-half jc-quads

    with (
        tc.tile_pool(name="big", bufs=1) as big,
        tc.tile_pool(name="small", bufs=1) as small,
        tc.tile_pool(name="mtp", bufs=6) as mtp,
        tc.tile_pool(name="osb", bufs=4) as osb,
        tc.tile_pool(name="psc", bufs=2, space="PSUM") as psc,
        tc.tile_pool(name="pso", bufs=1, space="PSUM") as pso,
    ):
        at8 = big.tile([128, 2 * IHW], F8, tag="at8")       # [p, ih, jc, 256]
        hh8 = big.tile([128, JC * D], F8, tag="hh8")        # h hi [p, jc, d]
        hl8 = big.tile([128, JC * D], F8, tag="hl8")        # h lo [p, jc, d]
        wt = small.tile([3, N], F16, tag="wt")              # W'.T
        w4 = small.tile([128, JC * 4], F16, tag="w4")       # W'|ones (j part)
        id16 = small.tile([128, 128], F16, tag="id16")
        scr = small.tile([128, 512], F16, tag="scr")        # warm-up source
        rN16 = small.tile([128, IC * H], F16, tag="rN16")   # 1/denom'
        rs32 = small.tile([128, IC], F32, tag="rs32")       # rowsum per ic
        rT16 = small.tile([3, SH], F16, tag="rT16")         # R'.T [k, i]

        psO = [
            pso.tile([128, D], F32, tag=f"psO{ic}", name=f"psO{ic}")
            for ic in range(IC)
        ]
        # R-chain scratch aliased into psO[2]/psO[3] (consumed before the
        # first B-half agg start wipes those banks)
        psD2 = psO[2][:, 0:16]                              # [128, 16] f32
        psRT = psO[3][0:3, 0:256].bitcast(F16)              # [3, 512] f16

        # ---------------- loads ----------------
        # sync/HWDGE queue: at8 A-half pieces (A denominators stream with
        # them), id/wt, first h-piece, at8 B-half pieces, remaining h pieces.
        # w4 on the scalar queue (needed by the first denominator matmuls).
        nc.scalar.dma_start(out=w4[:], in_=w4_in)
        AP_PIECES = [12, 12, 7, 1]

        a8_r = a8_in.rearrange("p (ih jc x) -> p ih jc x", ih=2, jc=JC)

        def at8_pieces(ih):
            off = 0
            bnds = []
            av = at8[:].rearrange("p (ih jc x) -> p ih jc x", ih=2, jc=JC)
            for n_ in AP_PIECES:
                nc.sync.dma_start(out=av[:, ih, off:off + n_],
                                  in_=a8_r[:, ih, off:off + n_])
                bnds.append((off, off + n_))
                off += n_
            return bnds

        bounds_a = at8_pieces(0)
        nc.sync.dma_start(out=id16[:], in_=id_in)
        nc.sync.dma_start(out=wt[:], in_=wt_in)
        hh_r = hh_in.rearrange("p (jc d) -> p jc d", d=D)
        hl_r = hl_in.rearrange("p (jc d) -> p jc d", d=D)
        hh_v = hh8[:].rearrange("p (jc d) -> p jc d", d=D)
        hl_v = hl8[:].rearrange("p (jc d) -> p jc d", d=D)

        def h_piece(j0, j1):
            nc.sync.dma_start(out=hh_v[:, j0:j1], in_=hh_r[:, j0:j1])
            nc.sync.dma_start(out=hl_v[:, j0:j1], in_=hl_r[:, j0:j1])

        h_piece(0, 4)
        bounds_b = at8_pieces(1)
        h_piece(4, 8)
        h_piece(8, 16)
        h_piece(16, 24)
        h_piece(24, 32)

        nc.vector.memset(scr[:], 0.0)
        # warm the ACT table (LoadActFuncSet) off the critical path
        actw = small.tile([1, 2], F16, tag="actw")
        nc.scalar.copy(actw[:], scr[0:1, 0:2])

        n_warm = [0]

        def warm(n_):
            # warm-up targets rotate over psO[0]/psO[1]: their garbage is
            # wiped by the first real agg matmul's start=True
            for _ in range(n_):
                nc.tensor.matmul(
                    psO[n_warm[0] % 2][:], scr[:, 0:128], scr[:],
                    start=True, stop=True, skip_group_check=True,
                )
                n_warm[0] += 1

        at8_v = at8[:].rearrange("p (ih jc x) -> p ih jc x", ih=2, jc=JC)
        den_state = {"first": True}

        def denoms(ih, j0, j1, last):
            # psD2[p_i, ic*4+k] += sum_j A[i,j] W'[j,k]; k=3 gives rowsum.
            # Single accumulation super-group across BOTH halves: start only
            # on the very first matmul (pending-zero covers the bank).
            for jc in range(j0, j1):
                for ii in range(2):
                    ic = ih * 2 + ii
                    nc.tensor.matmul(
                        psD2[:, ic * 4:(ic + 1) * 4],
                        at8_v[:, ih, jc, ii * 128:(ii + 1) * 128],
                        w4[:, jc * 4:(jc + 1) * 4],
                        start=den_state["first"],
                        stop=last and (jc == j1 - 1 and ii == 1),
                        skip_group_check=True,
                    )
                    den_state["first"] = False

        def r_chain(ih):
            # R' = 1/denom' (fp16) for this half, transposed to [k, i].
            # rowsum is folded into the psO stores later.
            psD2_v = psD2.rearrange("p (ic s) -> p ic s", s=4)
            with nc.allow_low_precision(reason="R' fits fp16"):
                nc.vector.reciprocal(
                    rN16[:, ih * 2 * H:(ih + 1) * 2 * H].rearrange(
                        "p (ic k) -> p ic k", k=H),
                    psD2_v[:, 2 * ih:2 * ih + 2, 0:H],
                )
            nc.vector.tensor_copy(
                rs32[:, 2 * ih:2 * ih + 2], psD2_v[:, 2 * ih:2 * ih + 2, 3]
            )
            for ii in range(2):
                ic = ih * 2 + ii
                nc.tensor.transpose(
                    psRT[:, ic * 128:(ic + 1) * 128],
                    rN16[:, ic * H:(ic + 1) * H],
                    id16[:],
                )
            nc.vector.tensor_copy(
                rT16[:, ih * 256:(ih + 1) * 256],
                psRT[:, ih * 256:(ih + 1) * 256],
            )

        # ---------------- sweep units ----------------
        # unit u = (ih, g): jc-quad 4g..4g+3 x i-half ih; pairs (2g, 2g+1)
        UNITS = [(0, 0), (0, 1), (0, 2), (0, 3), (0, 4),
                 (1, 0), (0, 5), (1, 1), (0, 6), (1, 2), (0, 7),
                 (1, 3), (1, 4), (1, 5), (1, 6), (1, 7)]
        cp_eng = ["act"] * NU
        sub_eng = [
            "dve" if (u % 5 == 0 or u >= NU - 3) else "pool"
            for u in range(NU)
        ]
        hh8_v = hh8[:].rearrange("p (pr two d) -> p pr two d", two=2, d=D)
        hl8_v = hl8[:].rearrange("p (pr two d) -> p pr two d", two=2, d=D)

        his = {}
        los = {}

        def front(u):
            ih, g = UNITS[u]
            ctp = psc.tile([128, 4 * 256], F32, tag="ctp", name=f"ctp{u}")
            for q in range(4):
                jc = 4 * g + q
                nc.tensor.matmul(
                    ctp[:, q * 256:(q + 1) * 256],
                    wt[0:3, jc * 128:(jc + 1) * 128],
                    rT16[:, ih * 256:(ih + 1) * 256],
                    start=True, stop=True,
                    tile_position=(0, 0),
                )
            mt16 = mtp.tile([128, 1024], F16, tag="mt16", name=f"mt16_{u}")
            nc.vector.tensor_tensor(
                mt16[:], at8_v[:, ih, 4 * g:4 * g + 4], ctp[:], op=mult,
            )
            hi8 = mtp.tile([128, 1024], F8, tag="hi8", name=f"hi8_{u}")
            lo8 = mtp.tile([128, 1024], F8, tag="lo8", name=f"lo8_{u}")
            if cp_eng[u] == "act":
                nc.scalar.copy(hi8[:], mt16[:])
            elif cp_eng[u] == "dve":
                nc.vector.tensor_copy(hi8[:], mt16[:])
            else:
                nc.gpsimd.tensor_copy(hi8[:], mt16[:])
            if sub_eng[u] == "dve":
                nc.vector.tensor_tensor(lo8[:], mt16[:], hi8[:], op=subop)
            else:
                nc.gpsimd.tensor_tensor(lo8[:], mt16[:], hi8[:], op=subop)
            his[u], los[u] = hi8, lo8

        # grouped stores: two ic per out-DMA; rowsum folds in as the scale.
        # Group 0 (ic0/1, A-half) completes mid-program and is fully hidden.
        ot2 = [
            osb.tile([128, 2 * D], F16, tag=f"ot{g}", name=f"ot{g}")
            for g in range(2)
        ]

        def store(ic):
            g, half = divmod(ic, 2)
            dst = ot2[g][:, half * D:(half + 1) * D]
            if ic % 2 == 0:
                nc.scalar.mul(dst, psO[ic][:], rs32[:, ic:ic + 1])
            else:
                nc.vector.tensor_scalar(
                    dst, psO[ic][:], rs32[:, ic:ic + 1], None, op0=mult
                )
            if half == 1:
                out_g = out[g * 256:(g + 1) * 256, :].rearrange(
                    "(two p) d -> p two d", two=2
                )
                (nc.sync if g == 0 else nc.scalar).dma_start(
                    out=out_g,
                    in_=ot2[g][:].rearrange("p (two d) -> p two d", two=2),
                )

        TERMS = ((0, 0), (0, 1), (1, 0))  # (hi/lo, hh/hl)
        started = set()
        last_u = {}                        # ih -> last unit index
        for u, (ih, g) in enumerate(UNITS):
            last_u[ih] = u

        def agg(u):
            ih, g = UNITS[u]
            hi8_v = his[u][:].rearrange("p (q x) -> p q x", q=4)
            lo8_v = los[u][:].rearrange("p (q x) -> p q x", q=4)
            lts = (hi8_v, lo8_v)
            rts = (hh8_v, hl8_v)
            final = last_u[ih] == u
            if final:
                order = [(ii, pr, t) for ii in range(2) for pr in range(2)
                         for t in range(3)]
            else:
                order = [(ii, pr, t) for t in range(3) for pr in range(2)
                         for ii in range(2)]
            for ii, pr, t in order:
                ic = 2 * ih + ii
                lt, rt = lts[TERMS[t][0]], rts[TERMS[t][1]]
                st = ic not in started
                started.add(ic)
                # lhsT: DR planes = the two jc of pair pr, i-slice ii
                lv = lt[:, 2 * pr:2 * pr + 2, ii * 128:(ii + 1) * 128]
                nc.tensor.matmul(
                    psO[ic][:],
                    lv,
                    rt[:, 2 * g + pr],
                    start=st,
                    stop=final and (pr == 1 and t == 2),
                    perf_mode=DR,
                    skip_group_check=True,
                )
                if final and pr == 1 and t == 2:
                    store(ic)

        # ---------------- emission schedule ----------------
        # PE in-order stream: warms + A-denominators stream with the A
        # pieces; A-half R chain; then the software-pipelined units with the
        # B denominators + B R chain sprinkled between early units so they
        # execute as the B pieces land without blocking A aggregations.
        warm(4)
        for pi, (j0, j1) in enumerate(bounds_a):
            denoms(0, j0, j1, False)
            if pi < len(bounds_a) - 2:
                warm(3)
        r_chain(0)

        LAG = 3
        emitted_b = [0]

        def maybe_emit_b(slot):
            # slot: how many units have been fronted so far
            if slot == 4 and emitted_b[0] == 0:
                denoms(1, *bounds_b[0], False)
                denoms(1, *bounds_b[1], False)
                emitted_b[0] = 2
            elif slot == 5 and emitted_b[0] == 2:
                denoms(1, *bounds_b[2], False)
                denoms(1, *bounds_b[3], True)
                r_chain(1)
                emitted_b[0] = 4

        for u in range(NU + LAG):
            if u < NU:
                front(u)
            maybe_emit_b(u + 1)
            if u >= LAG:
                agg(u - LAG)


_CACHE = {}


def _build1():
    if "p1" in _CACHE:
        return _CACHE["p1"]
    nc = bacc.Bacc("TRN2", target_bir_lowering=False, debug=False,
                   num_devices=NCORES)
    hst_in = nc.dram_tensor("hst_in", [128, IC * DC * 128], F16,
                            kind="ExternalInput").ap()
    p_in = nc.dram_tensor("p_in", [128, DC * H], F16, kind="ExternalInput").ap()
    w_out = nc.dram_tensor("w_out", [128, IC * H], F16,
                           kind="ExternalOutput").ap()
    with tile.TileContext(nc) as tc:
        _body1(tc, hst_in, p_in, w_out)
    nc.compile()
    _CACHE["p1"] = nc
    return nc


def _build2():
    if "p2" in _CACHE:
        return _CACHE["p2"]
    nc = bacc.Bacc("TRN2", target_bir_lowering=False, debug=False,
                   num_devices=NCORES)
    a8_in = nc.dram_tensor("a8_in", [128, JC * SH], F8,
                           kind="ExternalInput").ap()
    hh_in = nc.dram_tensor("hh_in", [128, JC * D], F8,
                           kind="ExternalInput").ap()
    hl_in = nc.dram_tensor("hl_in", [128, JC * D], F8,
                           kind="ExternalInput").ap()
    wt_in = nc.dram_tensor("wt_in", [3, N], F16, kind="ExternalInput").ap()
    w4_in = nc.dram_tensor("w4_in", [128, JC * 4], F16,
                           kind="ExternalInput").ap()
    id_in = nc.dram_tensor("id_in", [128, 128], F16, kind="ExternalInput").ap()
    out = nc.dram_tensor("out", [SH, D], F16, kind="ExternalOutput").ap()
    with tile.TileContext(nc) as tc:
        _body2(tc, a8_in, hh_in, hl_in, wt_in, w4_in, id_in, out)
    nc.compile()
    _CACHE["p2"] = nc
    return nc


def kernel(graph_info, h, P, _trace=False, _results_out=None):
    graph_info = np.ascontiguousarray(graph_info, dtype=np.float32)
    h = np.ascontiguousarray(h, dtype=np.float32)
    P = np.ascontiguousarray(P, dtype=np.float32)
    nc1 = _build1()
    nc2 = _build2()

    # host-side shard/layout prep (pure data movement + dtype casts)
    h16_full = h.astype(np.float16)
    p16_host = np.ascontiguousarray(
        P.astype(np.float16).reshape(DC, 128, H).transpose(1, 0, 2)
    ).reshape(128, DC * H)
    in1 = []
    for c in range(NCORES):
        hsT = h16_full[c * SH:(c + 1) * SH, :].T  # [D, SH]
        hst_host = np.ascontiguousarray(
            hsT.reshape(DC, 128, IC, 128).transpose(1, 2, 0, 3)
        ).reshape(128, IC * DC * 128)
        in1.append({"hst_in": hst_host, "p_in": p16_host})
    res1 = bass_utils.run_bass_kernel_spmd(
        nc1, in1, core_ids=list(range(NCORES)), trace=_trace
    )
    w_full = np.concatenate(
        [
            res1.results[c]["w_out"]
            .reshape(128, IC, H).transpose(1, 0, 2).reshape(SH, H)
            for c in range(NCORES)
        ],
        axis=0,
    )  # [N, 3] fp16, scaled by 2^-4

    wt_host = np.ascontiguousarray(w_full.T)  # [3, N]
    w4_host = np.ascontiguousarray(
        np.concatenate(
            [w_full.reshape(JC, 128, H).transpose(1, 0, 2),
             np.ones((128, JC, 1), np.float16)],
            axis=2,
        ).reshape(128, JC * 4)
    )
    id_host = np.eye(128, dtype=np.float16)

    # fp8 hi/lo split of h (host-side re-encoding; h = hh + hl up to e4m3^2)
    h_hi = np.clip(h, -240, 240).astype(NP_F8)
    h_lo = (h - h_hi.astype(np.float32)).astype(NP_F8)
    hh_host = np.ascontiguousarray(
        h_hi.reshape(JC, 128, D).transpose(1, 0, 2)).reshape(128, JC * D)
    hl_host = np.ascontiguousarray(
        h_lo.reshape(JC, 128, D).transpose(1, 0, 2)).reshape(128, JC * D)

    in2 = []
    for c in range(NCORES):
        at = np.ascontiguousarray(
            graph_info[c * SH:(c + 1) * SH, :].T
        ).astype(NP_F8)                      # [N(j), SH(i)]
        # pack [p, ihalf, jc, 256]: i-half-major so the A-half is contiguous
        a8_host = np.ascontiguousarray(
            at.reshape(JC, 128, 2, 256).transpose(1, 2, 0, 3)
        ).reshape(128, JC * SH)
        in2.append({
            "a8_in": a8_host,
            "hh_in": hh_host,
            "hl_in": hl_host,
            "wt_in": wt_host,
            "w4_in": w4_host,
            "id_in": id_host,
        })
    res2 = bass_utils.run_bass_kernel_spmd(
        nc2, in2, core_ids=list(range(NCORES)), trace=_trace
    )
    if _results_out is not None:
        _results_out.extend([res1, res2])
    return np.concatenate(
        [res2.results[c]["out"].astype(np.float32) for c in range(NCORES)],
        axis=0,
    )


# revision 85
# speedup vs baseline: 1.0954x; 1.0370x over previous
"""GAT-style attention (gnn_message_passing) Trainium2 kernel, 8-core row-parallel.

Math (algebraically identical to the reference masked-softmax attention):
  E = relu(h @ P)                 [N,3]
  W' = max(exp(E - 4ln2), 1/16)   (= exp(relu(E))/16, fp16-safe range)
  denom'[i,k] = sum_j A[i,j] W'[j,k]   (k=3 slot sums ones -> rowsum[i])
  R'[i,k] = rowsum[i] / denom'[i,k]
  ct[j,i]  = sum_k W'[j,k] R'[i,k] = rowsum[i] * C[i,j]
  mt[j,i]  = A[i,j] * ct[j,i]
  out[i,:] = sum_j mt[j,i] h[j,:]

Two SPMD programs (cost-modeled collectives are ~15us fixed -> too slow; the
tiny [4096,3] W matrix crosses cores via a host gather between programs):
  P1 (per core): W'-shard [512,3] from host-transposed h-shard (fp16 — fp8
      h.T fails the error budget through the exponential).
  host: concat W'-shards; build wt [3,N] / w4 (W'|ones) layouts; cast
      A-shard.T to fp8 (binary, exact) packed [p, ihalf, jc, 256]; split h
      into fp8 hi/lo halves (h = h_hi + h_lo, each e4m3 re-encoding).
  P2 (per core): 16 uniform [128,1024] sweep units (ihalf x jc-quad; see
      _body2's docstring). A-half units need only the first 1 MB of A.T,
      so aggregation starts ~3us earlier than a full-R schedule. Per unit:
        ct (4 fp16 matmuls into a [128,1024] PSUM slab)
        mt16 = at8 * ct          (DVE, the only full-size PSUM touch)
        mt_hi8 = fp8(mt16)       (ACT copies)
        mt_lo8 = mt16 - mt_hi8   (mostly POOL, DVE for drain-critical units)
        psO[ic] += DoubleRow fp8 matmuls: mt_hi.T@h_hi + mt_hi.T@h_lo
                   + mt_lo.T@h_hi   (3-term hi/lo split => 0.5% rel err,
                   4x cheaper than fp16 at the 0.5 cycles/row DR rate)
      Denominators stream with the A.T DMA pieces; 1/denom' stays unscaled
      and the rowsum factor folds into the final psO stores' scale. Grouped
      2-ic output DMAs; warm-up matmuls defeat the PE clock ramp.
"""

import numpy as np
import ml_dtypes

import concourse.bass as bass
import concourse.mybir as mybir
import concourse.tile as tile
from concourse import bacc
from concourse import bass_utils

N = 4096
D = 512
H = 3
NCORES = 8
SH = N // NCORES          # 512 output rows per core
JC = N // 128             # 32 j-chunks
IC = SH // 128            # 4 i-chunks
DC = D // 128             # 4 d-chunks
NP = JC // 2              # 16 jc-pairs
F8 = mybir.dt.float8e4
F16 = mybir.dt.float16
F32 = mybir.dt.float32
LN2x4 = float(4.0 * np.log(2.0))   # W scaled by 2^-4 to stay in fp16 range
NP_F8 = ml_dtypes.float8_e4m3
DR = mybir.MatmulPerfMode.DoubleRow


def _body1(tc, hst_in, p_in, w_out):
    """P1: W'-shard [SH,3] from hst [128, IC*DC*128] (h-shard.T, jc-major:
    hst[:, jc, dc, :] = h.T d-chunk dc for j-chunk jc), loaded in 2 pieces.
    The E matmuls use hst as the stationary operand (3-column streams)."""
    nc = tc.nc
    with (
        tc.tile_pool(name="sb1", bufs=1) as sb,
        tc.tile_pool(name="ps1", bufs=1, space="PSUM") as ps,
    ):
        hst = sb.tile([128, IC * DC * 128], F16, tag="hst")
        p16 = sb.tile([128, DC * H], F16, tag="p16")
        wsE = sb.tile([128, IC * H], F16, tag="wsE")
        ebias = sb.tile([128, 1], F32, tag="ebias")
        nc.gpsimd.dma_start(out=p16[:], in_=p_in)
        nc.vector.memset(ebias[:], -LN2x4)
        hst_v = hst[:].rearrange("p (g x) -> g p x", g=2)
        hin_v = hst_in.rearrange("p (g x) -> g p x", g=2)
        for g in range(2):
            nc.sync.dma_start(out=hst_v[g], in_=hin_v[g])

        # one PSUM tile spanning 4 banks: E group per jc, single exp at the end
        psE = ps.tile([128, IC * 512], F32, tag="psE", name="psE")
        for jc in range(IC):
            for dc in range(DC):
                nc.tensor.matmul(
                    psE[:, jc * 512: jc * 512 + H],
                    hst[:, (jc * DC + dc) * 128: (jc * DC + dc + 1) * 128],
                    p16[:, dc * H:(dc + 1) * H],
                    start=(dc == 0),
                    stop=(dc == DC - 1),
                )
        nc.scalar.activation(
            wsE[:].rearrange("p (jc k) -> p jc k", k=H),
            psE[:].rearrange("p (jc x) -> p jc x", x=512)[:, :, 0:H],
            mybir.ActivationFunctionType.Exp,
            bias=ebias[:], scale=1.0,
        )
        nc.vector.tensor_scalar_max(wsE[:], wsE[:], 0.0625)
        nc.sync.dma_start(out=w_out, in_=wsE[:])


def _body2(tc, a8_in, hh_in, hl_in, wt_in, w4_in, id_in, out):
    """P2: denominators + R' chain + 16 uniform sweep units with 3-term fp8
    DoubleRow aggregation.

    a8_in is A-shard.T fp8 packed [p, ihalf, jc, 256]: the i (output-row)
    space is split into two halves of 256. A unit = (ihalf, jc-quad): a
    [128, 1024] slab covering 4 j-chunks x one i-half = 2 DR pair-planes x
    2 ic blocks. A-half units depend only on the A-half denominators (the
    first 1 MB of at8), so the sweep starts ~3us earlier than a full-R
    schedule; B-half units start once the full at8 has landed.

    PSUM budget trick: the denominator accumulator psD2 and the R-transpose
    scratch psRT live inside psO[2]/psO[3]'s banks (bitcast slices). They
    are fully consumed before the first B-half aggregation's start=True
    wipes those banks.
    """
    nc = tc.nc
    mult = mybir.AluOpType.mult
    subop = mybir.AluOpType.subtract
    IHW = JC * 256                # bytes per i-half in at8's free dim
    NU = 16                       # units: 8 A-half + 8 B-half jc-quads

    with (
        tc.tile_pool(name="big", bufs=1) as big,
        tc.tile_pool(name="small", bufs=1) as small,
        tc.tile_pool(name="mtp", bufs=8) as mtp,
        tc.tile_pool(name="osb", bufs=4) as osb,
        tc.tile_pool(name="psc", bufs=2, space="PSUM") as psc,
        tc.tile_pool(name="pso", bufs=1, space="PSUM") as pso,
    ):
        at8 = big.tile([128, 2 * IHW], F8, tag="at8")       # [p, ih, jc, 256]
        hh8 = big.tile([128, JC * D], F8, tag="hh8")        # h hi [p, jc, d]
        hl8 = big.tile([128, JC * D], F8, tag="hl8")        # h lo [p, jc, d]
        wt = small.tile([3, N], F16, tag="wt")              # W'.T
        w4 = small.tile([128, JC * 4], F16, tag="w4")       # W'|ones (j part)
        id16 = small.tile([128, 128], F16, tag="id16")
        scr = small.tile([128, 512], F16, tag="scr")        # warm-up source
        rN16 = small.tile([128, IC * H], F16, tag="rN16")   # 1/denom'
        rs32 = small.tile([128, IC], F32, tag="rs32")       # rowsum per ic
        rT16 = small.tile([3, SH], F16, tag="rT16")         # R'.T [k, i]

        psO = [
            pso.tile([128, D], F32, tag=f"psO{ic}", name=f"psO{ic}")
            for ic in range(IC)
        ]
        # R-chain scratch aliased into psO[2]/psO[3] (consumed before the
        # first B-half agg start wipes those banks)
        psD2 = psO[2][:, 0:16]                              # [128, 16] f32
        psRT = psO[3][0:3, 0:256].bitcast(F16)              # [3, 512] f16

        # ---------------- loads ----------------
        # sync/HWDGE queue: at8 A-half pieces (A denominators stream with
        # them), id/wt, first h-piece, at8 B-half pieces, remaining h pieces.
        # w4 on the scalar queue (needed by the first denominator matmuls).
        nc.scalar.dma_start(out=w4[:], in_=w4_in)
        AP_PIECES = {0: [12, 12, 7, 1], 1: [16, 15, 1]}

        a8_r = a8_in.rearrange("p (ih jc x) -> p ih jc x", ih=2, jc=JC)

        def at8_pieces(ih):
            off = 0
            bnds = []
            av = at8[:].rearrange("p (ih jc x) -> p ih jc x", ih=2, jc=JC)
            for n_ in AP_PIECES[ih]:
                nc.sync.dma_start(out=av[:, ih, off:off + n_],
                                  in_=a8_r[:, ih, off:off + n_])
                bnds.append((off, off + n_))
                off += n_
            return bnds

        bounds_a = at8_pieces(0)
        nc.sync.dma_start(out=id16[:], in_=id_in)
        nc.sync.dma_start(out=wt[:], in_=wt_in)
        hh_r = hh_in.rearrange("p (jc d) -> p jc d", d=D)
        hl_r = hl_in.rearrange("p (jc d) -> p jc d", d=D)
        hh_v = hh8[:].rearrange("p (jc d) -> p jc d", d=D)
        hl_v = hl8[:].rearrange("p (jc d) -> p jc d", d=D)

        def h_piece(j0, j1):
            nc.sync.dma_start(out=hh_v[:, j0:j1], in_=hh_r[:, j0:j1])
            nc.sync.dma_start(out=hl_v[:, j0:j1], in_=hl_r[:, j0:j1])

        h_piece(0, 4)
        bounds_b = at8_pieces(1)
        h_piece(4, 8)
        h_piece(8, 12)
        h_piece(12, 16)
        h_piece(16, 24)
        h_piece(24, 32)

        nc.vector.memset(scr[:], 0.0)
        # warm the ACT table (LoadActFuncSet) off the critical path
        actw = small.tile([1, 2], F16, tag="actw")
        nc.scalar.copy(actw[:], scr[0:1, 0:2])

        n_warm = [0]

        def warm(n_):
            # warm-up targets rotate over psO[0]/psO[1]: their garbage is
            # wiped by the first real agg matmul's start=True
            for _ in range(n_):
                nc.tensor.matmul(
                    psO[n_warm[0] % 2][:], scr[:, 0:128], scr[:],
                    start=True, stop=True, skip_group_check=True,
                )
                n_warm[0] += 1

        at8_v = at8[:].rearrange("p (ih jc x) -> p ih jc x", ih=2, jc=JC)
        den_state = {"first": True}

        def denoms(ih, j0, j1, last):
            # psD2[p_i, ic*4+k] += sum_j A[i,j] W'[j,k]; k=3 gives rowsum.
            # Single accumulation super-group across BOTH halves: start only
            # on the very first matmul (pending-zero covers the bank).
            for jc in range(j0, j1):
                for ii in range(2):
                    ic = ih * 2 + ii
                    nc.tensor.matmul(
                        psD2[:, ic * 4:(ic + 1) * 4],
                        at8_v[:, ih, jc, ii * 128:(ii + 1) * 128],
                        w4[:, jc * 4:(jc + 1) * 4],
                        start=den_state["first"],
                        stop=last and (jc == j1 - 1 and ii == 1),
                        skip_group_check=True,
                    )
                    den_state["first"] = False

        def r_recip(ih):
            # R' = 1/denom' (fp16) for this half; rowsum kept for the stores
            psD2_v = psD2.rearrange("p (ic s) -> p ic s", s=4)
            with nc.allow_low_precision(reason="R' fits fp16"):
                nc.vector.reciprocal(
                    rN16[:, ih * 2 * H:(ih + 1) * 2 * H].rearrange(
                        "p (ic k) -> p ic k", k=H),
                    psD2_v[:, 2 * ih:2 * ih + 2, 0:H],
                )
            nc.vector.tensor_copy(
                rs32[:, 2 * ih:2 * ih + 2], psD2_v[:, 2 * ih:2 * ih + 2, 3]
            )

        def r_transpose(ih):
            for ii in range(2):
                ic = ih * 2 + ii
                nc.tensor.transpose(
                    psRT[:, ic * 128:(ic + 1) * 128],
                    rN16[:, ic * H:(ic + 1) * H],
                    id16[:],
                )
            nc.vector.tensor_copy(
                rT16[:, ih * 256:(ih + 1) * 256],
                psRT[:, ih * 256:(ih + 1) * 256],
            )

        def r_chain(ih):
            r_recip(ih)
            r_transpose(ih)

        # ---------------- sweep units ----------------
        # unit u = (ih, g): jc-quad 4g..4g+3 x i-half ih; pairs (2g, 2g+1)
        UNITS = [(0, 0), (0, 1), (0, 2), (0, 3), (0, 4),
                 (1, 0), (0, 5), (1, 1), (0, 6), (1, 2), (0, 7),
                 (1, 3), (1, 4), (1, 5), (1, 6), (1, 7)]
        cp_eng = ["act"] * NU
        cp_eng[0] = "dve"
        sub_eng = [
            "dve" if (u % 4 == 0 or u >= NU - 2) else "pool"
            for u in range(NU)
        ]
        hh8_v = hh8[:].rearrange("p (pr two d) -> p pr two d", two=2, d=D)
        hl8_v = hl8[:].rearrange("p (pr two d) -> p pr two d", two=2, d=D)

        his = {}
        los = {}

        def front(u):
            ih, g = UNITS[u]
            ctp = psc.tile([128, 4 * 256], F32, tag="ctp", name=f"ctp{u}")
            for q in range(4):
                jc = 4 * g + q
                nc.tensor.matmul(
                    ctp[:, q * 256:(q + 1) * 256],
                    wt[0:3, jc * 128:(jc + 1) * 128],
                    rT16[:, ih * 256:(ih + 1) * 256],
                    start=True, stop=True,
                    tile_position=(0, 0),
                )
            mt16 = mtp.tile([128, 1024], F16, tag="mt16", name=f"mt16_{u}")
            hi8 = mtp.tile([128, 1024], F8, tag="hi8", name=f"hi8_{u}")
            lo8 = mtp.tile([128, 1024], F8, tag="lo8", name=f"lo8_{u}")
            # unit 0's chain is the pipeline-fill critical path: run it in
            # two 512-halves so the pr=0 aggregation matmuls (which read
            # only the first half) start one half-chain earlier.
            halves = ((0, 1024),)
            for lo_c, hi_c in halves:
                qs = slice(4 * g + lo_c // 256, 4 * g + hi_c // 256)
                sl = slice(lo_c, hi_c)
                nc.vector.tensor_tensor(
                    mt16[:, sl], at8_v[:, ih, qs], ctp[:, sl], op=mult,
                )
                if cp_eng[u] == "act":
                    nc.scalar.copy(hi8[:, sl], mt16[:, sl])
                elif cp_eng[u] == "dve":
                    nc.vector.tensor_copy(hi8[:, sl], mt16[:, sl])
                else:
                    nc.gpsimd.tensor_copy(hi8[:, sl], mt16[:, sl])
                if sub_eng[u] == "dve":
                    nc.vector.tensor_tensor(
                        lo8[:, sl], mt16[:, sl], hi8[:, sl], op=subop)
                else:
                    nc.gpsimd.tensor_tensor(
                        lo8[:, sl], mt16[:, sl], hi8[:, sl], op=subop)
            his[u], los[u] = hi8, lo8

        # grouped stores: two ic per out-DMA; rowsum folds in as the scale.
        # Group 0 (ic0/1, A-half) completes mid-program and is fully hidden.
        ot2 = [
            osb.tile([128, 2 * D], F16, tag=f"ot{g}", name=f"ot{g}")
            for g in range(2)
        ]

        def store(ic):
            g, half = divmod(ic, 2)
            dst = ot2[g][:, half * D:(half + 1) * D]
            nc.scalar.mul(dst, psO[ic][:], rs32[:, ic:ic + 1])
            if half == 1:
                out_g = out[g * 256:(g + 1) * 256, :].rearrange(
                    "(two p) d -> p two d", two=2
                )
                nc.sync.dma_start(
                    out=out_g,
                    in_=ot2[g][:].rearrange("p (two d) -> p two d", two=2),
                )

        TERMS = ((0, 0), (0, 1), (1, 0))  # (hi/lo, hh/hl)
        started = set()
        last_u = {}                        # ih -> last unit index
        for u, (ih, g) in enumerate(UNITS):
            last_u[ih] = u

        def agg(u):
            ih, g = UNITS[u]
            hi8_v = his[u][:].rearrange("p (q x) -> p q x", q=4)
            lo8_v = los[u][:].rearrange("p (q x) -> p q x", q=4)
            lts = (hi8_v, lo8_v)
            rts = (hh8_v, hl8_v)
            final = last_u[ih] == u
            if final:
                order = [(ii, pr, t) for ii in range(2) for pr in range(2)
                         for t in range(3)]
            else:
                order = [(ii, pr, t) for t in range(3) for pr in range(2)
                         for ii in range(2)]
            for ii, pr, t in order:
                ic = 2 * ih + ii
                lt, rt = lts[TERMS[t][0]], rts[TERMS[t][1]]
                st = ic not in started
                started.add(ic)
                # lhsT: DR planes = the two jc of pair pr, i-slice ii
                lv = lt[:, 2 * pr:2 * pr + 2, ii * 128:(ii + 1) * 128]
                nc.tensor.matmul(
                    psO[ic][:],
                    lv,
                    rt[:, 2 * g + pr],
                    start=st,
                    stop=final and (pr == 1 and t == 2),
                    perf_mode=DR,
                    skip_group_check=True,
                )
                if final and pr == 1 and t == 2:
                    store(ic)

        # ---------------- emission schedule ----------------
        # PE in-order stream: warms + A-denominators stream with the A
        # pieces; A-half R chain; then the software-pipelined units with the
        # B denominators + B R chain sprinkled between early units so they
        # execute as the B pieces land without blocking A aggregations.
        warm(4)
        for pi, (j0, j1) in enumerate(bounds_a):
            denoms(0, j0, j1, False)
            if pi < len(bounds_a) - 2:
                warm(3)
        r_chain(0)

        LAG = 4
        emitted_b = [0]

        def maybe_emit_b(slot):
            # slot: how many units have been fronted so far
            if slot == 1 and emitted_b[0] == 0:
                for bi, bb in enumerate(bounds_b):
                    denoms(1, *bb, bi == len(bounds_b) - 1)
                r_recip(1)
                emitted_b[0] = 2
            elif slot == 2 and emitted_b[0] == 2:
                r_transpose(1)
                emitted_b[0] = 4

        for u in range(NU + LAG):
            if u < NU:
                front(u)
            maybe_emit_b(u + 1)
            if u >= LAG:
                agg(u - LAG)


_CACHE = {}


def _build1():
    if "p1" in _CACHE:
        return _CACHE["p1"]
    nc = bacc.Bacc("TRN2", target_bir_lowering=False, debug=False,
                   num_devices=NCORES)
    hst_in = nc.dram_tensor("hst_in", [128, IC * DC * 128], F16,
                            kind="ExternalInput").ap()
    p_in = nc.dram_tensor("p_in", [128, DC * H], F16, kind="ExternalInput").ap()
    w_out = nc.dram_tensor("w_out", [128, IC * H], F16,
                           kind="ExternalOutput").ap()
    with tile.TileContext(nc) as tc:
        _body1(tc, hst_in, p_in, w_out)
    nc.compile()
    _CACHE["p1"] = nc
    return nc


def _build2():
    if "p2" in _CACHE:
        return _CACHE["p2"]
    nc = bacc.Bacc("TRN2", target_bir_lowering=False, debug=False,
                   num_devices=NCORES)
    a8_in = nc.dram_tensor("a8_in", [128, JC * SH], F8,
                           kind="ExternalInput").ap()
    hh_in = nc.dram_tensor("hh_in", [128, JC * D], F8,
                           kind="ExternalInput").ap()
    hl_in = nc.dram_tensor("hl_in", [128, JC * D], F8,
                           kind="ExternalInput").ap()
    wt_in = nc.dram_tensor("wt_in", [3, N], F16, kind="ExternalInput").ap()
    w4_in = nc.dram_tensor("w4_in", [128, JC * 4], F16,
                           kind="ExternalInput").ap()
    id_in = nc.dram_tensor("id_in", [128, 128], F16, kind="ExternalInput").ap()
    out = nc.dram_tensor("out", [SH, D], F16, kind="ExternalOutput").ap()
    with tile.TileContext(nc) as tc:
        _body2(tc, a8_in, hh_in, hl_in, wt_in, w4_in, id_in, out)
    nc.compile()
    _CACHE["p2"] = nc
    return nc


def kernel(graph_info, h, P, _trace=False, _results_out=None):
    graph_info = np.ascontiguousarray(graph_info, dtype=np.float32)
    h = np.ascontiguousarray(h, dtype=np.float32)
    P = np.ascontiguousarray(P, dtype=np.float32)
    nc1 = _build1()
    nc2 = _build2()

    # host-side shard/layout prep (pure data movement + dtype casts)
    h16_full = h.astype(np.float16)
    p16_host = np.ascontiguousarray(
        P.astype(np.float16).reshape(DC, 128, H).transpose(1, 0, 2)
    ).reshape(128, DC * H)
    in1 = []
    for c in range(NCORES):
        hsT = h16_full[c * SH:(c + 1) * SH, :].T  # [D, SH]
        hst_host = np.ascontiguousarray(
            hsT.reshape(DC, 128, IC, 128).transpose(1, 2, 0, 3)
        ).reshape(128, IC * DC * 128)
        in1.append({"hst_in": hst_host, "p_in": p16_host})
    res1 = bass_utils.run_bass_kernel_spmd(
        nc1, in1, core_ids=list(range(NCORES)), trace=_trace
    )
    w_full = np.concatenate(
        [
            res1.results[c]["w_out"]
            .reshape(128, IC, H).transpose(1, 0, 2).reshape(SH, H)
            for c in range(NCORES)
        ],
        axis=0,
    )  # [N, 3] fp16, scaled by 2^-4

    wt_host = np.ascontiguousarray(w_full.T)  # [3, N]
    w4_host = np.ascontiguousarray(
        np.concatenate(
            [w_full.reshape(JC, 128, H).transpose(1, 0, 2),
             np.ones((128, JC, 1), np.float16)],
            axis=2,
        ).reshape(128, JC * 4)
    )
    id_host = np.eye(128, dtype=np.float16)

    # fp8 hi/lo split of h (host-side re-encoding; h = hh + hl up to e4m3^2)
    h_hi = np.clip(h, -240, 240).astype(NP_F8)
    h_lo = (h - h_hi.astype(np.float32)).astype(NP_F8)
    hh_host = np.ascontiguousarray(
        h_hi.reshape(JC, 128, D).transpose(1, 0, 2)).reshape(128, JC * D)
    hl_host = np.ascontiguousarray(
        h_lo.reshape(JC, 128, D).transpose(1, 0, 2)).reshape(128, JC * D)

    in2 = []
    for c in range(NCORES):
        at = np.ascontiguousarray(
            graph_info[c * SH:(c + 1) * SH, :].T
        ).astype(NP_F8)                      # [N(j), SH(i)]
        # pack [p, ihalf, jc, 256]: i-half-major so the A-half is contiguous
        a8_host = np.ascontiguousarray(
            at.reshape(JC, 128, 2, 256).transpose(1, 2, 0, 3)
        ).reshape(128, JC * SH)
        in2.append({
            "a8_in": a8_host,
            "hh_in": hh_host,
            "hl_in": hl_host,
            "wt_in": wt_host,
            "w4_in": w4_host,
            "id_in": id_host,
        })
    res2 = bass_utils.run_bass_kernel_spmd(
        nc2, in2, core_ids=list(range(NCORES)), trace=_trace
    )
    if _results_out is not None:
        _results_out.extend([res1, res2])
    return np.concatenate(
        [res2.results[c]["out"].astype(np.float32) for c in range(NCORES)],
        axis=0,
    )
